# revision 21
# baseline (speedup 1.0000x reference)
"""Trainium2 Bass kernel for nn_Attention_944892805701.

Dense transformer attention layer: QKV projection + RoPE + causal GQA SDPA +
output projection. B=2, S=2048, DIM=4096, 32 Q heads / 8 KV heads, hd=128.

Sharding (8 cores): 2 (batch) x 4 (head groups). Core (b, g) computes global
Q heads [8g, 8g+8) / KV heads [2g, 2g+2) of batch b and the corresponding
partial output projection y_partial = att_heads @ Wo[:, o_slice]^T. The host
sums the 4 head-group partials per batch.

Per-core device program, tuned for continuous TensorE occupancy (bf16
matmuls, N=512 moving operand, ~259ns/MM steady state):

  Projections (per 512-wide s-chunk): one head per PSUM slot, 32
    accumulating matmuls; RoPE applied on VectorE in bf16 during the drain
    (sign-folded sin table -> 4 DVE ops), vT transposed to natural [s, hd]
    tiles on the PE (128x128 transpose mode).

  Attention (per q-chunk, per head): scoresT = kT_tile x qT_chunk in
    [k-part, q-free] layout, exp on ScalarE (1/sqrt(hd) folded into the
    activation scale), causality via restricted column ranges plus one
    triangular mask multiply per diagonal block, denominator = bf16 DVE
    accumulation + ones-matmul partition reduce + ScalarE reciprocal +
    gpsimd partition broadcast, PV accumulated in PSUM and drained
    UNNORMALIZED (bf16) to persistent SBUF tiles, normalized in place.

  Two-chunk-deep software pipeline: attention of chunk c interleaves the
    projection matmuls of chunk c+2 as small "filler" units between its
    items, so every tensor attention reads has been ready for a full window
    (~100us) and the TensorE queue always has independent work while
    attention waits on ScalarE exp results. Attention of chunks 2/3
    interleaves the output projection of chunks 0/1+2; chunk 3's output
    projection runs as a dense tail. All weight tensors are pre-rearranged
    on the host so every DMA is a contiguous per-partition copy, and loads
    are issued a half-window ahead of first use.

Output per core: outT [4096, 2048] bf16 = y_partial^T; host transposes+sums.
"""

import math
from contextlib import ExitStack

import numpy as np
import ml_dtypes

import concourse.bass as bass  # noqa: F401
import concourse.tile as tile
from concourse import bacc, mybir
from concourse.bass_utils import run_bass_kernel_spmd

F32 = mybir.dt.float32
BF16 = mybir.dt.bfloat16

N_CORES = 8
DIM = 4096
N_HEADS = 32
N_KV_HEADS = 8
HEAD_DIM = 128
SEQ = 2048

HQ = N_HEADS // 4      # 8 local q heads
HKV = N_KV_HEADS // 4  # 2 local kv heads

SC = 512
P = 128


def build_program(seq=SEQ, dim=DIM, hq=HQ, hkv=HKV):
    nrep = hq // hkv
    nch = seq // SC
    ndt = dim // P
    nkt_total = seq // P
    dq = hq * HEAD_DIM
    dkv = hkv * HEAD_DIM
    scale = 1.0 / math.sqrt(HEAD_DIM)

    nc = bacc.Bacc("TRN2", target_bir_lowering=False, debug=False,
                   num_devices=N_CORES)

    xTd = nc.dram_tensor("xT", [dim, seq], BF16, kind="ExternalInput").ap()
    # host-rearranged weights: every load is a contiguous per-partition copy
    wqr = nc.dram_tensor("wqr", [hq // 2, ndt // 8, P, 8, 2 * HEAD_DIM],
                         BF16, kind="ExternalInput").ap()
    wkr = nc.dram_tensor("wkr", [ndt // 8, P, 8, dkv], BF16,
                         kind="ExternalInput").ap()
    wvr = nc.dram_tensor("wvr", [ndt // 8, P, 8, dkv], BF16,
                         kind="ExternalInput").ap()
    wotr = nc.dram_tensor("wotr", [dim // P, P, dq], BF16,
                          kind="ExternalInput").ap()
    cosT = nc.dram_tensor("cosT", [HEAD_DIM, seq], BF16,
                          kind="ExternalInput").ap()
    sinsT = nc.dram_tensor("sinsT", [HEAD_DIM, seq], BF16,
                           kind="ExternalInput").ap()
    tri = nc.dram_tensor("tri", [P, P], BF16, kind="ExternalInput").ap()
    iden = nc.dram_tensor("iden", [P, P], BF16, kind="ExternalInput").ap()
    ones_col = nc.dram_tensor("ones_col", [P, 1], BF16,
                              kind="ExternalInput").ap()
    outT = nc.dram_tensor("outT", [dim, seq], BF16, kind="ExternalOutput").ap()

    with ExitStack() as ctx:
        tc = ctx.enter_context(tile.TileContext(nc))
        ws = ctx.enter_context(tc.tile_pool(name="ws", bufs=4))     # f32 512
        wsb = ctx.enter_context(tc.tile_pool(name="wsb", bufs=52))  # bf16 512
        big = ctx.enter_context(tc.tile_pool(name="big",
                                             bufs=hq + hkv + 2))
        vp = ctx.enter_context(tc.tile_pool(name="vp", bufs=hkv * nkt_total))
        xtp = ctx.enter_context(tc.tile_pool(name="xtp", bufs=11))
        wqp = ctx.enter_context(tc.tile_pool(name="wqp", bufs=4))
        wkvp = ctx.enter_context(tc.tile_pool(name="wkvp", bufs=6))
        wop = ctx.enter_context(tc.tile_pool(name="wop", bufs=3))
        cns = ctx.enter_context(tc.tile_pool(name="cns", bufs=1))
        ps_s = ctx.enter_context(tc.tile_pool(name="ps_s", bufs=3,
                                              space="PSUM"))
        ps_o = ctx.enter_context(tc.tile_pool(name="ps_o", bufs=2,
                                              space="PSUM"))
        ps_p = ctx.enter_context(tc.tile_pool(name="ps_p", bufs=2,
                                              space="PSUM"))
        ps_d = ctx.enter_context(tc.tile_pool(name="ps_d", bufs=1,
                                              space="PSUM"))

        mneg_sb = cns.tile([P, P], BF16, tag="tri")
        iden_sb = cns.tile([P, P], BF16, tag="iden")
        ones_sb = cns.tile([P, 1], BF16, tag="ones")
        # rope tables, resident for the whole kernel. sins has the sign of
        # the rotate-half fold baked in: rows [0,64) = -sin, [64,128) = +sin.
        cos_f = big.tile([P, seq], BF16, tag="big", name="cos_f")
        sins_f = big.tile([P, seq], BF16, tag="big", name="sins_f")

        def emit_tables():
            # emitted after the first weight/x loads so they don't delay
            # the first matmuls in the DMA queue
            nc.sync.dma_start(mneg_sb[:], tri[:])
            nc.sync.dma_start(iden_sb[:], iden[:])
            nc.sync.dma_start(ones_sb[:], ones_col[:])
            nc.sync.dma_start(cos_f[:], cosT[:])
            nc.sync.dma_start(sins_f[:], sinsT[:])

        qTr = [big.tile([P, seq], BF16, tag="big", name=f"qTr{i}")
               for i in range(hq)]
        kTr = [big.tile([P, seq], BF16, tag="big", name=f"kTr{i}")
               for i in range(hkv)]
        v_nat = [[vp.tile([P, HEAD_DIM], BF16, tag="v", name=f"v{g}_{t}")
                  for t in range(nkt_total)] for g in range(hkv)]
        # unnormalized attention output tiles, persistent until out-proj
        ao = [[wsb.tile([P, SC], BF16, tag="wsb", name=f"ao{h}_{cc}")
               for cc in range(nch)] for h in range(hq)]

        def emit_loads(c, tables_hook=None):
            """Merged contiguous DMA loads for chunk c: 8 xT tiles of
            [128,4,512], 16 wq + 4 wk + 4 wv tiles of [128,8,256]."""
            s0 = c * SC
            t = {"xT": [], "wq": {}, "wk": [], "wv": []}
            # interleave first xT tiles with the first wq tiles so both
            # streams start promptly
            split = 4 if c == 0 else 1  # fine-grained first loads: the very
            # first matmul waits on these, so smaller pieces start it sooner
            for dt4p in range(ndt // 8):
                for half in range(2):
                    dt4 = dt4p * 2 + half
                    xt = xtp.tile([P, 4, SC], BF16, tag="xt",
                                  name=f"xT{c}_{dt4}")
                    xq = nc.sync if half == 0 else nc.scalar
                    for sp in range(split):
                        d0, d1 = sp * 4 // split, (sp + 1) * 4 // split
                        xq.dma_start(
                            xt[:, d0:d1, :],
                            xTd[(dt4 * 4 + d0) * P:(dt4 * 4 + d1) * P,
                                s0:s0 + SC
                                ].rearrange("(d p) f -> p d f", p=P))
                    t["xT"].append(xt)
                wq = wqp.tile([P, 8, 2 * HEAD_DIM], BF16, tag="wq",
                              name=f"wq{c}_0_{dt4p}")
                wqq = nc.scalar if (c == 0 and dt4p == 0) else nc.sync
                for sp in range(split):
                    d0, d1 = sp * 8 // split, (sp + 1) * 8 // split
                    wqq.dma_start(wq[:, d0:d1, :],
                                  wqr[0, dt4p, :, d0:d1, :])
                t["wq"][(0, dt4p)] = wq
                split = max(1, split // 2) if c == 0 else 1
                if dt4p == 0 and tables_hook is not None:
                    tables_hook()
            for hb in range(2, hq, 2):
                for dt4p in range(ndt // 8):
                    wq = wqp.tile([P, 8, 2 * HEAD_DIM], BF16, tag="wq",
                                  name=f"wq{c}_{hb}_{dt4p}")
                    nc.sync.dma_start(wq[:], wqr[hb // 2, dt4p])
                    t["wq"][(hb, dt4p)] = wq
            for dt4p in range(ndt // 8):
                wk = wkvp.tile([P, 8, dkv], BF16, tag="wkv",
                               name=f"wk{c}_{dt4p}")
                nc.scalar.dma_start(wk[:], wkr[dt4p])
                t["wk"].append(wk)
            for dt4p in range(ndt // 8):
                wv = wkvp.tile([P, 8, dkv], BF16, tag="wkv",
                               name=f"wv{c}_{dt4p}")
                nc.scalar.dma_start(wv[:], wvr[dt4p])
                t["wv"].append(wv)
            return t

        def proj_units(c, t):
            """Generator: emits the chunk-c projections in small units,
            yielding between units so attention items can interleave.

            One head per PSUM slot, 32 accumulating matmuls each; with 3
            slots in the pool a drain has ~3 head-groups (~25us) of runway
            before its slot is reallocated, so drain latency never stalls
            the PE."""
            s0 = c * SC
            xT = t["xT"]
            h2 = HEAD_DIM // 2
            cos_c = cos_f[:, s0:s0 + SC]
            sins_c = sins_f[:, s0:s0 + SC]

            def rope_drain(dst, psum):
                # dst = psum*cos + rotate_half(psum)*sin, bf16 out.
                # Cross-half reads come from PSUM (exempt from the SBUF
                # same-start-partition rule); sin sign baked into the table.
                tmp = wsb.tile([P, SC], BF16, tag="wsb")
                nc.vector.tensor_mul(tmp[0:h2, :], psum[h2:P, :],
                                     sins_c[0:h2, :])
                nc.vector.tensor_mul(tmp[h2:P, :], psum[0:h2, :],
                                     sins_c[h2:P, :])
                nc.vector.tensor_mul(dst, psum[:], cos_c)
                nc.vector.tensor_add(dst, dst, tmp[:])

            def mm_groups(wsel):
                """32 accumulating matmuls into one fresh psum tile, in
                units of 8 (one merged weight tile each)."""
                ps = ps_p.tile([P, SC], F32, tag="p")

                def gen():
                    for dt4p in range(ndt // 8):
                        w = wsel(dt4p)
                        for jj in range(8):
                            dt = dt4p * 8 + jj
                            nc.tensor.matmul(
                                ps[:], w[:, jj, :],
                                xT[dt // 4][:, dt % 4, :],
                                start=(dt == 0), stop=(dt == ndt - 1))
                        yield
                return ps, gen

            for hh in range(hq):
                hb, i = (hh // 2) * 2, hh % 2
                pq, gen = mm_groups(
                    lambda dt4p, hb=hb, i=i:
                    t["wq"][(hb, dt4p)][:, :,
                                        i * HEAD_DIM:(i + 1) * HEAD_DIM])
                for _ in gen():
                    yield
                rope_drain(qTr[hh][:, s0:s0 + SC], pq[:])
                yield
            for g in range(hkv):
                pk, gen = mm_groups(
                    lambda dt4p, g=g:
                    t["wk"][dt4p][:, :, g * HEAD_DIM:(g + 1) * HEAD_DIM])
                for _ in gen():
                    yield
                rope_drain(kTr[g][:, s0:s0 + SC], pk[:])
                yield
            for g in range(hkv):
                pv, gen = mm_groups(
                    lambda dt4p, g=g:
                    t["wv"][dt4p][:, :, g * HEAD_DIM:(g + 1) * HEAD_DIM])
                for _ in gen():
                    yield
                vt = wsb.tile([P, SC], BF16, tag="wsb")
                nc.vector.tensor_copy(vt[:], pv[:])
                yield
                yield
                yield
                for st in range(SC // P):
                    pt = ps_p.tile([P, P], BF16, tag="p")
                    nc.tensor.transpose(pt[:], vt[:, st * P:(st + 1) * P],
                                        iden_sb[:])
                    nc.vector.tensor_copy(v_nat[g][c * (SC // P) + st][:],
                                          pt[:])
                yield

        N_PROJ_UNITS = (hq + hkv) * (ndt // 8 + 1) + hkv * (ndt // 8 + 4)

        def outproj_units(cc_list):
            """Generator: output projection outT[m,:] = sum_o WoT.T @ ao for
            the given chunks, in small units."""
            for m in range(dim // P):
                wo = wop.tile([P, dq], BF16, tag="wo")
                ldq = nc.scalar if m % 2 == 0 else nc.sync
                stq = nc.sync if m % 2 == 0 else nc.scalar
                ldq.dma_start(wo[:], wotr[m])
                yield
                for cc in cc_list:
                    py = ps_p.tile([P, SC], F32, tag="p")
                    for ob in range(0, hq, 4):
                        for o in range(ob, ob + 4):
                            nc.tensor.matmul(
                                py[:], wo[:, o * P:(o + 1) * P], ao[o][cc][:],
                                start=(o == 0), stop=(o == hq - 1))
                        yield
                    yo = wsb.tile([P, SC], BF16, tag="wsb")
                    nc.scalar.copy(yo[:], py[:])
                    stq.dma_start(
                        outT[m * P:(m + 1) * P, cc * SC:(cc + 1) * SC],
                        yo[:])
                    yield

        def n_outproj_units(cc_list):
            return (dim // P) * (1 + len(cc_list) * (hq // 4 + 1))

        def emit_denorm(c2, h, acc):
            """Denominator reduce + reciprocal + broadcast + in-place
            normalize of ao[h][c2].

            The DVE reciprocal is an iterative divide (~8 cyc per element
            per lane, ~3.3us for the 512-wide row), so it is split into 4
            ops to keep the DVE FIFO from blocking attention's elementwise
            work behind it."""
            pd = ps_d.tile([P, SC], F32, tag="d", name=f"pd{c2}_{h}")
            nc.tensor.matmul(pd[0:1, :], ones_sb[:], acc[:],
                             start=True, stop=True)
            rrow = ws.tile([P, SC], F32, tag="ws", name=f"rr{c2}_{h}")
            for q4 in range(4):
                nc.vector.reciprocal(rrow[0:1, q4 * P:(q4 + 1) * P],
                                     pd[0:1, q4 * P:(q4 + 1) * P])
            rb = ws.tile([P, SC], F32, tag="ws", name=f"rb{c2}_{h}")
            nc.gpsimd.partition_broadcast(rb[:], rrow[0:1, :])
            nc.vector.tensor_mul(ao[h][c2][:], ao[h][c2][:], rb[:])

        class FillerQueue:
            """Filler units spill across attention windows instead of
            force-draining at each window end."""

            def __init__(self):
                self.q = []

            def push(self, gen):
                self.q.append(gen)

            def remaining_hint(self, n):
                self._hint = n

            def step(self):
                while self.q:
                    try:
                        next(self.q[0])
                        return True
                    except StopIteration:
                        self.q.pop(0)
                return False

            def drain_through(self, gen):
                # emit everything up to and including gen (correctness
                # barrier: e.g. proj(c) must be fully emitted before
                # att(c) items reference its outputs)
                while self.q:
                    g = self.q.pop(0)
                    for _ in g:
                        pass
                    if g is gen:
                        return

            def drain_all(self):
                while self.q:
                    g = self.q.pop(0)
                    for _ in g:
                        pass

        def emit_attention(c, filler=None, n_units=0, mid_hook=None,
                           start_delay=0):
            s0 = c * SC
            nkt = (c + 1) * (SC // P)

            def emit_scores(h, g, kt):
                rr = kt * P - s0
                jlo = max(0, rr)
                diag = rr >= 0
                pscr = ps_s.tile([P, SC], F32, tag="s",
                                 name=f"pscr{c}_{h}_{kt}")
                nc.tensor.matmul(
                    pscr[:, jlo:SC],
                    kTr[g][:, kt * P:(kt + 1) * P],
                    qTr[h][:, s0 + jlo:s0 + SC],
                    start=True, stop=not diag)
                if diag:
                    # causal mask: add tril(-2^30, -1) onto the diagonal
                    # block in PSUM (iden.T @ mneg = mneg), so exp -> 0 and
                    # no elementwise mask op is needed
                    nc.tensor.matmul(
                        pscr[:, jlo:jlo + P], iden_sb[:], mneg_sb[:],
                        start=False, stop=True)
                return pscr

            items = [(h, kt) for h in range(hq) for kt in range(nkt)]
            nit = max(1, len(items) - start_delay)
            rate = (n_units / nit) if filler is not None else 0.0
            credit = [-rate * start_delay]

            def step_filler():
                if filler is None:
                    return
                credit[0] += rate
                while credit[0] >= 1.0:
                    if not filler.step():
                        return
                    credit[0] -= 1.0

            # flat (h, kt) stream with scores emitted 2 ahead across
            # head boundaries; denominators deferred into the next head
            pipe = {}

            def sc_ahead(i):
                h2, kt2 = items[i]
                pipe[i] = emit_scores(h2, h2 // nrep, kt2)

            sc_ahead(0)
            if len(items) > 1:
                sc_ahead(1)
            pending = None
            accs = {}
            pos = {}
            mid = len(items) // 2
            for i, (h, kt) in enumerate(items):
                if i == mid and mid_hook is not None:
                    mid_hook()
                step_filler()
                g = h // nrep
                if kt == 0:
                    accs[h] = wsb.tile([P, SC], BF16, tag="wsb",
                                       name=f"acc{c}_{h}")
                    pos[h] = ps_o.tile([P, SC], F32, tag="o",
                                       name=f"po{c}_{h}")
                acc, po = accs[h], pos[h]
                rr = kt * P - s0
                jlo = max(0, rr)
                if i + 2 < len(items):
                    sc_ahead(i + 2)
                pscr = pipe.pop(i)
                if kt == 2 and pending is not None:
                    emit_denorm(*pending)
                    pending = None
                et = wsb.tile([P, SC], BF16, tag="wsb",
                              name=f"et{c}_{h}_{kt}")
                nc.scalar.activation(
                    et[:, jlo:SC], pscr[:, jlo:SC],
                    mybir.ActivationFunctionType.Exp, scale=scale)
                if kt == 0:
                    nc.vector.tensor_copy(acc[:], et[:])
                else:
                    nc.vector.tensor_add(acc[:, jlo:SC],
                                         acc[:, jlo:SC], et[:, jlo:SC])
                nc.tensor.matmul(
                    po[:, jlo:SC],
                    v_nat[g][kt][:],
                    et[:, jlo:SC],
                    start=(kt == 0), stop=(kt == nkt - 1))
                if kt == nkt - 1:
                    if pending is not None:
                        emit_denorm(*pending)
                    pending = (c, h, accs[h])
                    nc.vector.tensor_copy(ao[h][c][:], po[:])
                    del accs[h], pos[h]
            if pending is not None:
                emit_denorm(*pending)

        # two-chunk-deep pipeline: proj(0)/proj(1) run dense upfront;
        # attention(c) interleaves proj(c+2), then the output projection of
        # earlier chunks; chunk 3's output projection is the dense tail.
        tl = {0: emit_loads(0, tables_hook=emit_tables),
              1: emit_loads(1)}
        for _ in proj_units(0, tl[0]):
            pass
        tl[2] = emit_loads(2)
        # hold back the tail of proj(1): DMA-independent filler that lets
        # attention(0) start at full PE rate while chunk-2 loads stream in
        HOLD = 14
        p1 = proj_units(1, tl[1])
        for _ in range(N_PROJ_UNITS - HOLD):
            next(p1)
        fq = FillerQueue()
        fq.push(p1)
        p2 = proj_units(2, tl[2])
        fq.push(p2)

        def _load3():
            tl[3] = emit_loads(3)

        emit_attention(0, fq, HOLD + N_PROJ_UNITS, mid_hook=_load3)
        p3 = proj_units(3, tl[3])
        fq.push(p3)
        fq.drain_through(p2)  # proj(2) complete before att(2) emission
        emit_attention(1, fq, N_PROJ_UNITS)
        fq.push(outproj_units([0]))
        fq.drain_through(p3)  # proj(3) complete before att(3) emission
        emit_attention(2, fq, n_outproj_units([0]))
        fq.push(outproj_units([1, 2]))
        emit_attention(3, fq, n_outproj_units([1, 2]), start_delay=4)
        fq.drain_all()
        # dense tail: output projection of the last chunk
        for _ in outproj_units([nch - 1]):
            pass

    nc.compile()
    return nc


def make_core_inputs(data, Wq, Wk, Wv, Wo, cos, sin):
    """Build in_maps for the 8 cores. Core id = 4*b + g."""
    bf = ml_dtypes.bfloat16

    def cbf(a):
        return np.ascontiguousarray(np.asarray(a).astype(bf))

    dq = HQ * HEAD_DIM
    dkv = HKV * HEAD_DIM
    dim = Wq.shape[1]
    ndt = dim // P
    tri_m = np.tril(np.full((P, P), -2.0**30, dtype=np.float32), -1).astype(bf)
    iden = np.eye(P, dtype=bf)
    ones_col = np.ones((P, 1), dtype=bf)
    cosT = cbf(cos.T)
    sinT = np.asarray(sin.T, dtype=np.float32).copy()
    h2 = HEAD_DIM // 2
    sinT[0:h2, :] *= -1.0  # rotate-half sign fold
    sinsT = cbf(sinT)
    xt_by_batch = [cbf(data[b].T) for b in range(data.shape[0])]

    def wr_blocks(wT, width):
        # wT [dim, width] -> [ndt//8, P, 8, width] with
        # out[b, p, d, f] = wT[b*1024 + d*128 + p, f]
        return np.ascontiguousarray(
            wT.reshape(ndt // 8, 8, P, width).transpose(0, 2, 1, 3))

    in_maps = []
    for core in range(N_CORES):
        b, g = divmod(core, 4)
        qs = slice(g * dq, (g + 1) * dq)
        ks = slice(g * dkv, (g + 1) * dkv)
        wqT = cbf(Wq[qs, :].T)                   # [dim, dq]
        wqr = np.stack([wr_blocks(wqT[:, hb * P:(hb + 2) * P], 2 * P)
                        for hb in range(0, HQ, 2)])  # [4, 4, P, 8, 256]
        wkr = wr_blocks(cbf(Wk[ks, :].T), dkv)
        wvr = wr_blocks(cbf(Wv[ks, :].T), dkv)
        woT = cbf(Wo[:, qs].T)                   # [dq, dim]
        wotr = np.ascontiguousarray(
            woT.reshape(HQ, P, dim // P, P).transpose(2, 1, 0, 3)
               .reshape(dim // P, P, dq))
        in_maps.append({
            "xT": xt_by_batch[b],
            "wqr": wqr,
            "wkr": wkr,
            "wvr": wvr,
            "wotr": wotr,
            "cosT": cosT,
            "sinsT": sinsT,
            "tri": tri_m,
            "iden": iden,
            "ones_col": ones_col,
        })
    return in_maps


_COMPILED = {}


def _get_program():
    key = (SEQ, DIM, HQ, HKV)
    if key not in _COMPILED:
        _COMPILED[key] = build_program()
    return _COMPILED[key]


def run(inputs, trace=False, tmpdir=None, trace_cores=None):
    nc = _get_program()
    in_maps = make_core_inputs(
        inputs["data"], inputs["Wq"], inputs["Wk"], inputs["Wv"],
        inputs["Wo"], inputs["cos"], inputs["sin"])
    kw = {}
    if trace:
        kw = dict(trace=True, tmpdir=tmpdir, trace_cores=trace_cores)
    res = run_bass_kernel_spmd(nc, in_maps, list(range(N_CORES)), **kw)
    B = inputs["data"].shape[0]
    out = np.zeros((B, SEQ, DIM), dtype=np.float32)
    for core in range(N_CORES):
        b = core // 4
        out[b] += res.results[core]["outT"].T.astype(np.float32)
    return out, res


def kernel(data, Wq, Wk, Wv, Wo, cos, sin, mask):
    assert np.asarray(mask).size == 1, "only causal (numel==1) mask supported"
    inputs = {
        "data": np.asarray(data, dtype=np.float32),
        "Wq": np.asarray(Wq, dtype=np.float32),
        "Wk": np.asarray(Wk, dtype=np.float32),
        "Wv": np.asarray(Wv, dtype=np.float32),
        "Wo": np.asarray(Wo, dtype=np.float32),
        "cos": np.asarray(cos, dtype=np.float32),
        "sin": np.asarray(sin, dtype=np.float32),
    }
    out, _ = run(inputs)
    return out


# revision 22
# speedup vs baseline: 1.1549x; 1.1549x over previous
"""Trainium2 Bass kernel for nn_Attention_944892805701.

Dense transformer attention layer: QKV projection + RoPE + causal GQA SDPA +
output projection. B=2, S=2048, DIM=4096, 32 Q heads / 8 KV heads, hd=128.

Sharding (8 cores): 2 (batch) x 4 (head groups). Core (b, g) computes global
Q heads [8g, 8g+8) / KV heads [2g, 2g+2) of batch b and the corresponding
partial output projection y_partial = att_heads @ Wo[:, o_slice]^T. The host
sums the 4 head-group partials per batch.

Per-core device program, tuned for continuous TensorE occupancy (bf16
matmuls, N=512 moving operand, ~259ns/MM steady state):

  Projections (per 512-wide s-chunk): one head per PSUM slot, 32
    accumulating matmuls; RoPE applied on VectorE in bf16 during the drain
    (sign-folded sin table -> 4 DVE ops), vT transposed to natural [s, hd]
    tiles on the PE (128x128 transpose mode).

  Attention (per q-chunk, per head): scoresT = kT_tile x qT_chunk in
    [k-part, q-free] layout, exp on ScalarE (1/sqrt(hd) folded into the
    activation scale), causality via restricted column ranges plus one
    triangular mask multiply per diagonal block, denominator = bf16 DVE
    accumulation + ones-matmul partition reduce + ScalarE reciprocal +
    gpsimd partition broadcast, PV accumulated in PSUM and drained
    UNNORMALIZED (bf16) to persistent SBUF tiles, normalized in place.

  Two-chunk-deep software pipeline: attention of chunk c interleaves the
    projection matmuls of chunk c+2 as small "filler" units between its
    items, so every tensor attention reads has been ready for a full window
    (~100us) and the TensorE queue always has independent work while
    attention waits on ScalarE exp results. Attention of chunks 2/3
    interleaves the output projection of chunks 0/1+2; chunk 3's output
    projection runs as a dense tail. All weight tensors are pre-rearranged
    on the host so every DMA is a contiguous per-partition copy, and loads
    are issued a half-window ahead of first use.

Output per core: outT [4096, 2048] bf16 = y_partial^T; host transposes+sums.
"""

import math
from contextlib import ExitStack

import numpy as np
import ml_dtypes

import concourse.bass as bass  # noqa: F401
import concourse.tile as tile
from concourse import bacc, mybir
from concourse.bass_utils import run_bass_kernel_spmd

F32 = mybir.dt.float32
BF16 = mybir.dt.bfloat16

N_CORES = 8
DIM = 4096
N_HEADS = 32
N_KV_HEADS = 8
HEAD_DIM = 128
SEQ = 2048

HQ = N_HEADS // 4      # 8 local q heads
HKV = N_KV_HEADS // 4  # 2 local kv heads

SC = 512
P = 128


def build_program(seq=SEQ, dim=DIM, hq=HQ, hkv=HKV):
    nrep = hq // hkv
    nch = seq // SC
    ndt = dim // P
    nkt_total = seq // P
    dq = hq * HEAD_DIM
    dkv = hkv * HEAD_DIM
    scale = 1.0 / math.sqrt(HEAD_DIM)

    nc = bacc.Bacc("TRN2", target_bir_lowering=False, debug=False,
                   num_devices=N_CORES)

    xTd = nc.dram_tensor("xT", [dim, seq], BF16, kind="ExternalInput").ap()
    # host-rearranged weights: every load is a contiguous per-partition copy
    wqr = nc.dram_tensor("wqr", [hq // 2, ndt // 8, P, 8, 2 * HEAD_DIM],
                         BF16, kind="ExternalInput").ap()
    wkr = nc.dram_tensor("wkr", [ndt // 8, P, 8, dkv], BF16,
                         kind="ExternalInput").ap()
    wvr = nc.dram_tensor("wvr", [ndt // 8, P, 8, dkv], BF16,
                         kind="ExternalInput").ap()
    wotr = nc.dram_tensor("wotr", [dim // P, P, dq], BF16,
                          kind="ExternalInput").ap()
    cosT = nc.dram_tensor("cosT", [HEAD_DIM, seq], BF16,
                          kind="ExternalInput").ap()
    sinsT = nc.dram_tensor("sinsT", [HEAD_DIM, seq], BF16,
                           kind="ExternalInput").ap()
    tri = nc.dram_tensor("tri", [P, P], BF16, kind="ExternalInput").ap()
    iden = nc.dram_tensor("iden", [P, P], BF16, kind="ExternalInput").ap()
    ones_col = nc.dram_tensor("ones_col", [P, 1], BF16,
                              kind="ExternalInput").ap()
    outT = nc.dram_tensor("outT", [dim, seq], BF16, kind="ExternalOutput").ap()

    with ExitStack() as ctx:
        tc = ctx.enter_context(tile.TileContext(nc))
        ws = ctx.enter_context(tc.tile_pool(name="ws", bufs=4))     # f32 512
        wsb = ctx.enter_context(tc.tile_pool(name="wsb", bufs=52))  # bf16 512
        big = ctx.enter_context(tc.tile_pool(name="big",
                                             bufs=hq + hkv + 2))
        vp = ctx.enter_context(tc.tile_pool(name="vp", bufs=hkv * nkt_total))
        xtp = ctx.enter_context(tc.tile_pool(name="xtp", bufs=11))
        wqp = ctx.enter_context(tc.tile_pool(name="wqp", bufs=4))
        wkvp = ctx.enter_context(tc.tile_pool(name="wkvp", bufs=6))
        wop = ctx.enter_context(tc.tile_pool(name="wop", bufs=3))
        cns = ctx.enter_context(tc.tile_pool(name="cns", bufs=1))
        ps_s = ctx.enter_context(tc.tile_pool(name="ps_s", bufs=3,
                                              space="PSUM"))
        ps_o = ctx.enter_context(tc.tile_pool(name="ps_o", bufs=2,
                                              space="PSUM"))
        ps_p = ctx.enter_context(tc.tile_pool(name="ps_p", bufs=2,
                                              space="PSUM"))
        ps_d = ctx.enter_context(tc.tile_pool(name="ps_d", bufs=1,
                                              space="PSUM"))

        mneg_sb = cns.tile([P, P], BF16, tag="tri")
        iden_sb = cns.tile([P, P], BF16, tag="iden")
        ones_sb = cns.tile([P, 1], BF16, tag="ones")
        # rope tables, resident for the whole kernel. sins has the sign of
        # the rotate-half fold baked in: rows [0,64) = -sin, [64,128) = +sin.
        cos_f = big.tile([P, seq], BF16, tag="big", name="cos_f")
        sins_f = big.tile([P, seq], BF16, tag="big", name="sins_f")

        def emit_tables():
            # emitted after the first weight/x loads so they don't delay
            # the first matmuls in the DMA queue
            nc.sync.dma_start(mneg_sb[:], tri[:])
            nc.sync.dma_start(iden_sb[:], iden[:])
            nc.sync.dma_start(ones_sb[:], ones_col[:])
            nc.sync.dma_start(cos_f[:], cosT[:])
            nc.sync.dma_start(sins_f[:], sinsT[:])

        qTr = [big.tile([P, seq], BF16, tag="big", name=f"qTr{i}")
               for i in range(hq)]
        kTr = [big.tile([P, seq], BF16, tag="big", name=f"kTr{i}")
               for i in range(hkv)]
        v_nat = [[vp.tile([P, HEAD_DIM], BF16, tag="v", name=f"v{g}_{t}")
                  for t in range(nkt_total)] for g in range(hkv)]
        # unnormalized attention output tiles, persistent until out-proj
        ao = [[wsb.tile([P, SC], BF16, tag="wsb", name=f"ao{h}_{cc}")
               for cc in range(nch)] for h in range(hq)]

        def emit_loads(c, tables_hook=None):
            """Merged contiguous DMA loads for chunk c: 8 xT tiles of
            [128,4,512], 16 wq + 4 wk + 4 wv tiles of [128,8,256]."""
            s0 = c * SC
            t = {"xT": [], "wq": {}, "wk": [], "wv": []}
            # interleave first xT tiles with the first wq tiles so both
            # streams start promptly
            split = 4 if c == 0 else 1  # fine-grained first loads: the very
            # first matmul waits on these, so smaller pieces start it sooner
            for dt4p in range(ndt // 8):
                for half in range(2):
                    dt4 = dt4p * 2 + half
                    xt = xtp.tile([P, 4, SC], BF16, tag="xt",
                                  name=f"xT{c}_{dt4}")
                    xq = nc.sync if half == 0 else nc.scalar
                    for sp in range(split):
                        d0, d1 = sp * 4 // split, (sp + 1) * 4 // split
                        xq.dma_start(
                            xt[:, d0:d1, :],
                            xTd[(dt4 * 4 + d0) * P:(dt4 * 4 + d1) * P,
                                s0:s0 + SC
                                ].rearrange("(d p) f -> p d f", p=P))
                    t["xT"].append(xt)
                wq = wqp.tile([P, 8, 2 * HEAD_DIM], BF16, tag="wq",
                              name=f"wq{c}_0_{dt4p}")
                for sp in range(split):
                    d0, d1 = sp * 8 // split, (sp + 1) * 8 // split
                    nc.sync.dma_start(wq[:, d0:d1, :],
                                      wqr[0, dt4p, :, d0:d1, :])
                t["wq"][(0, dt4p)] = wq
                split = max(1, split // 2) if c == 0 else 1
                if dt4p == 0 and tables_hook is not None:
                    tables_hook()
            for hb in range(2, hq, 2):
                for dt4p in range(ndt // 8):
                    wq = wqp.tile([P, 8, 2 * HEAD_DIM], BF16, tag="wq",
                                  name=f"wq{c}_{hb}_{dt4p}")
                    nc.sync.dma_start(wq[:], wqr[hb // 2, dt4p])
                    t["wq"][(hb, dt4p)] = wq
            for dt4p in range(ndt // 8):
                wk = wkvp.tile([P, 8, dkv], BF16, tag="wkv",
                               name=f"wk{c}_{dt4p}")
                nc.scalar.dma_start(wk[:], wkr[dt4p])
                t["wk"].append(wk)
            for dt4p in range(ndt // 8):
                wv = wkvp.tile([P, 8, dkv], BF16, tag="wkv",
                               name=f"wv{c}_{dt4p}")
                nc.scalar.dma_start(wv[:], wvr[dt4p])
                t["wv"].append(wv)
            return t

        def proj_units(c, t):
            """Generator: emits the chunk-c projections in small units,
            yielding between units so attention items can interleave.

            One head per PSUM slot, 32 accumulating matmuls each; with 3
            slots in the pool a drain has ~3 head-groups (~25us) of runway
            before its slot is reallocated, so drain latency never stalls
            the PE."""
            s0 = c * SC
            xT = t["xT"]
            h2 = HEAD_DIM // 2
            cos_c = cos_f[:, s0:s0 + SC]
            sins_c = sins_f[:, s0:s0 + SC]

            def rope_drain(dst, psum):
                # dst = psum*cos + rotate_half(psum)*sin, bf16 out.
                # Cross-half reads come from PSUM (exempt from the SBUF
                # same-start-partition rule); sin sign baked into the table.
                tmp = wsb.tile([P, SC], BF16, tag="wsb")
                nc.vector.tensor_mul(tmp[0:h2, :], psum[h2:P, :],
                                     sins_c[0:h2, :])
                nc.vector.tensor_mul(tmp[h2:P, :], psum[0:h2, :],
                                     sins_c[h2:P, :])
                nc.vector.tensor_mul(dst, psum[:], cos_c)
                nc.vector.tensor_add(dst, dst, tmp[:])

            def mm_groups(wsel):
                """32 accumulating matmuls into one fresh psum tile, in
                units of 8 (one merged weight tile each)."""
                ps = ps_p.tile([P, SC], F32, tag="p")

                def gen():
                    for dt4p in range(ndt // 8):
                        w = wsel(dt4p)
                        for jj in range(8):
                            dt = dt4p * 8 + jj
                            nc.tensor.matmul(
                                ps[:], w[:, jj, :],
                                xT[dt // 4][:, dt % 4, :],
                                start=(dt == 0), stop=(dt == ndt - 1))
                        yield
                return ps, gen

            for hh in range(hq):
                hb, i = (hh // 2) * 2, hh % 2
                pq, gen = mm_groups(
                    lambda dt4p, hb=hb, i=i:
                    t["wq"][(hb, dt4p)][:, :,
                                        i * HEAD_DIM:(i + 1) * HEAD_DIM])
                for _ in gen():
                    yield
                rope_drain(qTr[hh][:, s0:s0 + SC], pq[:])
                yield
            for g in range(hkv):
                pk, gen = mm_groups(
                    lambda dt4p, g=g:
                    t["wk"][dt4p][:, :, g * HEAD_DIM:(g + 1) * HEAD_DIM])
                for _ in gen():
                    yield
                rope_drain(kTr[g][:, s0:s0 + SC], pk[:])
                yield
            for g in range(hkv):
                pv, gen = mm_groups(
                    lambda dt4p, g=g:
                    t["wv"][dt4p][:, :, g * HEAD_DIM:(g + 1) * HEAD_DIM])
                for _ in gen():
                    yield
                vt = wsb.tile([P, SC], BF16, tag="wsb")
                nc.vector.tensor_copy(vt[:], pv[:])
                yield
                yield
                yield
                for st in range(SC // P):
                    pt = ps_p.tile([P, P], BF16, tag="p")
                    nc.tensor.transpose(pt[:], vt[:, st * P:(st + 1) * P],
                                        iden_sb[:])
                    nc.vector.tensor_copy(v_nat[g][c * (SC // P) + st][:],
                                          pt[:])
                yield

        N_PROJ_UNITS = (hq + hkv) * (ndt // 8 + 1) + hkv * (ndt // 8 + 4)

        def outproj_units(cc_list):
            """Generator: output projection outT[m,:] = sum_o WoT.T @ ao for
            the given chunks, in small units."""
            for m in range(dim // P):
                wo = wop.tile([P, dq], BF16, tag="wo")
                ldq = nc.scalar if m % 2 == 0 else nc.sync
                stq = nc.sync if m % 2 == 0 else nc.scalar
                ldq.dma_start(wo[:], wotr[m])
                yield
                for cc in cc_list:
                    py = ps_p.tile([P, SC], F32, tag="p")
                    for ob in range(0, hq, 4):
                        for o in range(ob, ob + 4):
                            nc.tensor.matmul(
                                py[:], wo[:, o * P:(o + 1) * P], ao[o][cc][:],
                                start=(o == 0), stop=(o == hq - 1))
                        yield
                    yo = wsb.tile([P, SC], BF16, tag="wsb")
                    nc.scalar.copy(yo[:], py[:])
                    stq.dma_start(
                        outT[m * P:(m + 1) * P, cc * SC:(cc + 1) * SC],
                        yo[:])
                    yield

        def n_outproj_units(cc_list):
            return (dim // P) * (1 + len(cc_list) * (hq // 4 + 1))

        def emit_denorm(c2, h, acc):
            """Denominator reduce + reciprocal + broadcast + in-place
            normalize of ao[h][c2].

            The DVE reciprocal is an iterative divide (~8 cyc per element
            per lane, ~3.3us for the 512-wide row), so it is split into 4
            ops to keep the DVE FIFO from blocking attention's elementwise
            work behind it."""
            pd = ps_d.tile([P, SC], F32, tag="d", name=f"pd{c2}_{h}")
            nc.tensor.matmul(pd[0:1, :], ones_sb[:], acc[:],
                             start=True, stop=True)
            rrow = ws.tile([P, SC], F32, tag="ws", name=f"rr{c2}_{h}")
            for q4 in range(4):
                nc.vector.reciprocal(rrow[0:1, q4 * P:(q4 + 1) * P],
                                     pd[0:1, q4 * P:(q4 + 1) * P])
            rb = ws.tile([P, SC], F32, tag="ws", name=f"rb{c2}_{h}")
            nc.gpsimd.partition_broadcast(rb[:], rrow[0:1, :])
            nc.vector.tensor_mul(ao[h][c2][:], ao[h][c2][:], rb[:])

        def emit_attention(c, filler=None, n_units=0, mid_hook=None,
                           start_delay=0):
            s0 = c * SC
            nkt = (c + 1) * (SC // P)

            def emit_scores(h, g, kt):
                rr = kt * P - s0
                jlo = max(0, rr)
                diag = rr >= 0
                pscr = ps_s.tile([P, SC], F32, tag="s",
                                 name=f"pscr{c}_{h}_{kt}")
                nc.tensor.matmul(
                    pscr[:, jlo:SC],
                    kTr[g][:, kt * P:(kt + 1) * P],
                    qTr[h][:, s0 + jlo:s0 + SC],
                    start=True, stop=not diag)
                if diag:
                    # causal mask: add tril(-2^30, -1) onto the diagonal
                    # block in PSUM (iden.T @ mneg = mneg), so exp -> 0 and
                    # no elementwise mask op is needed
                    nc.tensor.matmul(
                        pscr[:, jlo:jlo + P], iden_sb[:], mneg_sb[:],
                        start=False, stop=True)
                return pscr

            items = [(h, kt) for h in range(hq) for kt in range(nkt)]
            nit = max(1, len(items) - start_delay)
            rate = (n_units / nit) if filler is not None else 0.0
            credit = [-rate * start_delay]
            exhausted = [filler is None]

            def step_filler():
                if exhausted[0]:
                    return
                credit[0] += rate
                while credit[0] >= 1.0:
                    try:
                        next(filler)
                    except StopIteration:
                        exhausted[0] = True
                        return
                    credit[0] -= 1.0

            # flat (h, kt) stream with scores emitted 2 ahead across
            # head boundaries; denominators deferred into the next head
            pipe = {}

            def sc_ahead(i):
                h2, kt2 = items[i]
                pipe[i] = emit_scores(h2, h2 // nrep, kt2)

            sc_ahead(0)
            if len(items) > 1:
                sc_ahead(1)
            pending = None
            accs = {}
            pos = {}
            mid = len(items) // 2
            for i, (h, kt) in enumerate(items):
                if i == mid and mid_hook is not None:
                    mid_hook()
                step_filler()
                g = h // nrep
                if kt == 0:
                    accs[h] = wsb.tile([P, SC], BF16, tag="wsb",
                                       name=f"acc{c}_{h}")
                    pos[h] = ps_o.tile([P, SC], F32, tag="o",
                                       name=f"po{c}_{h}")
                acc, po = accs[h], pos[h]
                rr = kt * P - s0
                jlo = max(0, rr)
                if i + 2 < len(items):
                    sc_ahead(i + 2)
                pscr = pipe.pop(i)
                if kt == 2 and pending is not None:
                    emit_denorm(*pending)
                    pending = None
                et = wsb.tile([P, SC], BF16, tag="wsb",
                              name=f"et{c}_{h}_{kt}")
                nc.scalar.activation(
                    et[:, jlo:SC], pscr[:, jlo:SC],
                    mybir.ActivationFunctionType.Exp, scale=scale)
                if kt == 0:
                    nc.vector.tensor_copy(acc[:], et[:])
                else:
                    nc.vector.tensor_add(acc[:, jlo:SC],
                                         acc[:, jlo:SC], et[:, jlo:SC])
                nc.tensor.matmul(
                    po[:, jlo:SC],
                    v_nat[g][kt][:],
                    et[:, jlo:SC],
                    start=(kt == 0), stop=(kt == nkt - 1))
                if kt == nkt - 1:
                    if pending is not None:
                        emit_denorm(*pending)
                    pending = (c, h, accs[h])
                    nc.vector.tensor_copy(ao[h][c][:], po[:])
                    del accs[h], pos[h]
            if pending is not None:
                emit_denorm(*pending)
            if filler is not None and not exhausted[0]:
                for _ in filler:
                    pass

        # two-chunk-deep pipeline: proj(0)/proj(1) run dense upfront;
        # attention(c) interleaves proj(c+2), then the output projection of
        # earlier chunks; chunk 3's output projection is the dense tail.
        tl = {0: emit_loads(0, tables_hook=emit_tables),
              1: emit_loads(1)}
        for _ in proj_units(0, tl[0]):
            pass
        tl[2] = emit_loads(2)
        for _ in proj_units(1, tl[1]):
            pass

        def _load3():
            tl[3] = emit_loads(3)

        emit_attention(0, proj_units(2, tl[2]), N_PROJ_UNITS,
                       mid_hook=_load3, start_delay=10)
        emit_attention(1, proj_units(3, tl[3]), N_PROJ_UNITS,
                       start_delay=10)
        emit_attention(2, outproj_units([0]), n_outproj_units([0]))
        emit_attention(3, outproj_units([1, 2]),
                       n_outproj_units([1, 2]), start_delay=6)
        # dense tail: output projection of the last chunk
        for _ in outproj_units([nch - 1]):
            pass

    nc.compile()
    return nc


def make_core_inputs(data, Wq, Wk, Wv, Wo, cos, sin):
    """Build in_maps for the 8 cores. Core id = 4*b + g."""
    bf = ml_dtypes.bfloat16

    def cbf(a):
        return np.ascontiguousarray(np.asarray(a).astype(bf))

    dq = HQ * HEAD_DIM
    dkv = HKV * HEAD_DIM
    dim = Wq.shape[1]
    ndt = dim // P
    tri_m = np.tril(np.full((P, P), -2.0**30, dtype=np.float32), -1).astype(bf)
    iden = np.eye(P, dtype=bf)
    ones_col = np.ones((P, 1), dtype=bf)
    cosT = cbf(cos.T)
    sinT = np.asarray(sin.T, dtype=np.float32).copy()
    h2 = HEAD_DIM // 2
    sinT[0:h2, :] *= -1.0  # rotate-half sign fold
    sinsT = cbf(sinT)
    xt_by_batch = [cbf(data[b].T) for b in range(data.shape[0])]

    def wr_blocks(wT, width):
        # wT [dim, width] -> [ndt//8, P, 8, width] with
        # out[b, p, d, f] = wT[b*1024 + d*128 + p, f]
        return np.ascontiguousarray(
            wT.reshape(ndt // 8, 8, P, width).transpose(0, 2, 1, 3))

    in_maps = []
    for core in range(N_CORES):
        b, g = divmod(core, 4)
        qs = slice(g * dq, (g + 1) * dq)
        ks = slice(g * dkv, (g + 1) * dkv)
        wqT = cbf(Wq[qs, :].T)                   # [dim, dq]
        wqr = np.stack([wr_blocks(wqT[:, hb * P:(hb + 2) * P], 2 * P)
                        for hb in range(0, HQ, 2)])  # [4, 4, P, 8, 256]
        wkr = wr_blocks(cbf(Wk[ks, :].T), dkv)
        wvr = wr_blocks(cbf(Wv[ks, :].T), dkv)
        woT = cbf(Wo[:, qs].T)                   # [dq, dim]
        wotr = np.ascontiguousarray(
            woT.reshape(HQ, P, dim // P, P).transpose(2, 1, 0, 3)
               .reshape(dim // P, P, dq))
        in_maps.append({
            "xT": xt_by_batch[b],
            "wqr": wqr,
            "wkr": wkr,
            "wvr": wvr,
            "wotr": wotr,
            "cosT": cosT,
            "sinsT": sinsT,
            "tri": tri_m,
            "iden": iden,
            "ones_col": ones_col,
        })
    return in_maps


_COMPILED = {}


def _get_program():
    key = (SEQ, DIM, HQ, HKV)
    if key not in _COMPILED:
        _COMPILED[key] = build_program()
    return _COMPILED[key]


def run(inputs, trace=False, tmpdir=None, trace_cores=None):
    nc = _get_program()
    in_maps = make_core_inputs(
        inputs["data"], inputs["Wq"], inputs["Wk"], inputs["Wv"],
        inputs["Wo"], inputs["cos"], inputs["sin"])
    kw = {}
    if trace:
        kw = dict(trace=True, tmpdir=tmpdir, trace_cores=trace_cores)
    res = run_bass_kernel_spmd(nc, in_maps, list(range(N_CORES)), **kw)
    B = inputs["data"].shape[0]
    out = np.zeros((B, SEQ, DIM), dtype=np.float32)
    for core in range(N_CORES):
        b = core // 4
        out[b] += res.results[core]["outT"].T.astype(np.float32)
    return out, res


def kernel(data, Wq, Wk, Wv, Wo, cos, sin, mask):
    assert np.asarray(mask).size == 1, "only causal (numel==1) mask supported"
    inputs = {
        "data": np.asarray(data, dtype=np.float32),
        "Wq": np.asarray(Wq, dtype=np.float32),
        "Wk": np.asarray(Wk, dtype=np.float32),
        "Wv": np.asarray(Wv, dtype=np.float32),
        "Wo": np.asarray(Wo, dtype=np.float32),
        "cos": np.asarray(cos, dtype=np.float32),
        "sin": np.asarray(sin, dtype=np.float32),
    }
    out, _ = run(inputs)
    return out
